# revision 8
# baseline (speedup 1.0000x reference)
"""Fused dequant-GEMM (quint8 affine) on 8 TRN2 NeuronCores.

out = ((x - 65) * 0.199) @ ((y - 160) * 0.0215),  x,y: [4096, 4096] uint8-valued int32.

Strategy (tensor-parallel, per sharding hint):
  - Shard y column-wise into 8 blocks of 512; replicate x. No collectives.
  - Host pre-packs both operands as zero-point-shifted bf16 (integers in
    [-160, 190] are exact in bf16), laid out so DMA lines are contiguous per
    partition and SBUF tiles are directly consumable as matmul operands
    (K on the partition axis).
  - Startup is latency-critical: a small throwaway matmul burst trips the PE
    HAM clock gate toward 2.4GHz while the first data DMAs land. The first 8
    m-tiles run k-major across all 8 PSUM banks, so the combined x+y stream
    is consumed at ~220 GB/s -- safely under the DMA ramp rate. x chunks
    (4 k-tiles each) are issued alternately on the sync and vector DMA
    queues; y chunks (geometric sizes) stream on the gpsimd queue.
  - Remaining 24 m-tiles run m-major: one 1MB x DMA per m-tile (8KB/partition
    lines), 32 accumulating matmuls into one PSUM bank, epilogue scale
    (0.199*0.0215) on alternating Scalar/Vector engines, DMA out.
  - Host concatenates the 8 [4096, 512] core outputs. Exactness: bf16 holds
    these integer ranges exactly; PE multiplies exactly and accumulates in
    fp32, so only fp32 rounding remains (~1e-7 vs the fp32 reference).
"""

import numpy as np
import ml_dtypes

M = 4096
K = 4096
N = 4096
NCORES = 8
P = 128
NSH = N // NCORES  # 512 columns per core
MT = M // P        # 32 m-tiles
KT = K // P        # 32 k-tiles

G = 8                          # m-tiles in the k-major startup group
XC0 = [2, 2, 4, 8, 16]         # k-tiles per x chunk within the startup group
YCH = [1, 1, 2, 4, 8, 8, 8]    # k-tiles per y chunk
N_WARMUP_MM = 7                # throwaway matmuls to trip the HAM clock gate

ZP_X = 65.0
ZP_Y = 160.0
# Match the reference's fp32 scale arithmetic as closely as possible.
SCALE = float(np.float32(0.199) * np.float32(0.0215))

_CACHE = {}


def build_nc():
    """Build + compile the per-core Bass graph (identical on all 8 cores)."""
    from concourse import bass, bacc, tile, mybir

    assert sum(YCH) == KT and sum(XC0) == KT

    nc = bacc.Bacc("TRN2", target_bir_lowering=False, debug=False)
    bf16 = mybir.dt.bfloat16
    f32 = mybir.dt.float32

    # x packed as [mt, p=k%128, kt*128+m] -> contiguous per partition row
    x_d = nc.dram_tensor("x", [MT, P, K], bf16, kind="ExternalInput").ap()
    # y shard packed as [p=k%128, kt*512+n], shipped as uint8 (half the
    # startup DMA bytes); converted to zero-point-shifted bf16 on VectorE.
    u8 = mybir.dt.uint8
    y_d = nc.dram_tensor("y", [P, KT * NSH], u8, kind="ExternalInput").ap()
    # out as [mt, m, n]
    o_d = nc.dram_tensor("out", [MT, P, NSH], f32, kind="ExternalOutput").ap()

    with tile.TileContext(nc) as tc:
        with (
            tc.tile_pool(name="wpool", bufs=1) as wpool,
            tc.tile_pool(name="ypool", bufs=1) as ypool,
            tc.tile_pool(name="x0pool", bufs=1) as x0pool,
            tc.tile_pool(name="xpool", bufs=4) as xpool,
            tc.tile_pool(name="opool", bufs=6) as opool,
            tc.tile_pool(name="ppool", bufs=8, space=bass.MemorySpace.PSUM) as ppool,
        ):
            # PE warm-up: one zeroed tile serves as both operands; the PSUM
            # bank is overwritten (start=True) by real work later and never
            # read meanwhile.
            w = wpool.tile([P, NSH], bf16, name="w")
            nc.gpsimd.memset(w[:], 0.0)
            wps = ppool.tile([P, NSH], f32, name="wps", tag="ps")
            for _ in range(N_WARMUP_MM):
                nc.tensor.matmul(wps[:], w[:, :P], w[:], start=True, stop=True)

            # y chunks (geometric sizes) on the gpsimd DMA queue; DVE does
            # the uint8 -> (v - 160) bf16 dequant (exact for these integers)
            y_ts = []
            base = 0
            for ci, ckt in enumerate(YCH):
                tu = ypool.tile([P, ckt * NSH], u8, name=f"yu{ci}", tag=f"yu{ci}")
                nc.gpsimd.dma_start(
                    tu[:], y_d[:, base * NSH:(base + ckt) * NSH]
                )
                t = ypool.tile([P, ckt * NSH], bf16, name=f"y{ci}", tag=f"y{ci}")
                nc.vector.tensor_scalar_add(t[:], tu[:], -ZP_Y)
                y_ts.append((base, ckt, t))
                base += ckt

            def y_slice(kt):
                for k0, ckt, t in y_ts:
                    if k0 <= kt < k0 + ckt:
                        return t[:, (kt - k0) * NSH:(kt - k0 + 1) * NSH]
                raise AssertionError(kt)

            # --- startup group: m-tiles 0..G-1, k-major across G PSUM banks ---
            xg0 = [[] for _ in range(G)]
            xbase = 0
            for ci, ckt in enumerate(XC0):
                for m in range(G):
                    t = x0pool.tile(
                        [P, ckt * P], bf16, name=f"x0_{m}_{ci}", tag=f"x0_{m}_{ci}"
                    )
                    eng = nc.sync if m % 2 == 0 else nc.scalar
                    eng.dma_start(
                        t[:], x_d[m][:, xbase * P:(xbase + ckt) * P]
                    )
                    xg0[m].append((xbase, ckt, t))
                xbase += ckt

            def x0_slice(m, kt):
                for k0, ckt, t in xg0[m]:
                    if k0 <= kt < k0 + ckt:
                        return t[:, (kt - k0) * P:(kt - k0 + 1) * P]
                raise AssertionError((m, kt))

            ps0 = [ppool.tile([P, NSH], f32, name="ps", tag="ps") for _ in range(G)]
            for kt in range(KT):
                for m in range(G):
                    nc.tensor.matmul(
                        ps0[m][:],
                        x0_slice(m, kt),
                        y_slice(kt),
                        start=(kt == 0),
                        stop=(kt == KT - 1),
                    )

            def epilogue(mt, ps_tile):
                o_t = opool.tile([P, NSH], f32, name="o_t", tag="o_t")
                if mt % 2 == 0:
                    nc.scalar.mul(o_t[:], ps_tile[:], SCALE)
                else:
                    nc.vector.tensor_scalar_mul(o_t[:], ps_tile[:], SCALE)
                nc.sync.dma_start(o_d[mt], o_t[:])

            for m in range(G):
                epilogue(m, ps0[m])

            # --- steady state: m-tiles G..MT-1, m-major ---
            for mt in range(G, MT):
                x_t = xpool.tile([P, K], bf16, name="x_t", tag="x_t")
                eng = nc.sync if mt % 2 == 0 else nc.scalar
                eng.dma_start(x_t[:], x_d[mt])
                ps = ppool.tile([P, NSH], f32, name="ps", tag="ps")
                for kt in range(KT):
                    nc.tensor.matmul(
                        ps[:],
                        x_t[:, kt * P:(kt + 1) * P],
                        y_slice(kt),
                        start=(kt == 0),
                        stop=(kt == KT - 1),
                    )
                epilogue(mt, ps)

    nc.compile()
    return nc


def prep_in_maps(x, y):
    """Shift zero-points, cast to bf16 (exact for these integer ranges), and
    pack for partition-contiguous DMA. Returns one in_map per core."""
    bf16 = ml_dtypes.bfloat16
    x = np.asarray(x)
    y = np.asarray(y)

    xd = (x.astype(np.float32) - np.float32(ZP_X)).astype(bf16)  # [M, K]
    # [mt, m, kt, p] -> [mt, p, kt, m]
    xp = np.ascontiguousarray(
        xd.reshape(MT, P, KT, P).transpose(0, 3, 2, 1)
    ).reshape(MT, P, K)

    # y ships as raw uint8; the kernel subtracts the zero point on-device
    yp = y.astype(np.uint8).reshape(KT, P, N).transpose(1, 0, 2)  # [p, kt, n]

    in_maps = []
    for c in range(NCORES):
        ysh = np.ascontiguousarray(yp[:, :, c * NSH:(c + 1) * NSH]).reshape(
            P, KT * NSH
        )
        in_maps.append({"x": xp, "y": ysh})
    return in_maps


def assemble_output(results):
    cols = [np.asarray(r["out"], dtype=np.float32).reshape(M, NSH) for r in results]
    return np.concatenate(cols, axis=1)


def get_nc():
    if "nc" not in _CACHE:
        _CACHE["nc"] = build_nc()
    return _CACHE["nc"]


def kernel(x, y):
    from concourse.bass_utils import run_bass_kernel_spmd

    nc = get_nc()
    in_maps = prep_in_maps(x, y)
    res = run_bass_kernel_spmd(nc, in_maps, core_ids=list(range(NCORES)))
    out = assemble_output(res.results)
    if np.isnan(out).any():
        # Cold-start insurance: a fresh device stack once produced NaN on the
        # very first execution; a retry has always been clean.
        res = run_bass_kernel_spmd(nc, in_maps, core_ids=list(range(NCORES)))
        out = assemble_output(res.results)
    return out


# revision 9
# speedup vs baseline: 1.1670x; 1.1670x over previous
"""Fused dequant-GEMM (quint8 affine) on 8 TRN2 NeuronCores.

out = ((x - 65) * 0.199) @ ((y - 160) * 0.0215),  x,y: [4096, 4096] uint8-valued int32.

Strategy (tensor-parallel, per sharding hint):
  - Shard y column-wise into 8 blocks of 512; replicate x. No collectives.
  - Host pre-packs both operands as zero-point-shifted bf16 (integers in
    [-160, 190] are exact in bf16), laid out so DMA lines are contiguous per
    partition and SBUF tiles are directly consumable as matmul operands
    (K on the partition axis).
  - Startup is latency-critical: a small throwaway matmul burst trips the PE
    HAM clock gate toward 2.4GHz while the first data DMAs land. The first 8
    m-tiles run k-major across all 8 PSUM banks, so the combined x+y stream
    is consumed at ~220 GB/s -- safely under the DMA ramp rate. x chunks
    (4 k-tiles each) are issued alternately on the sync and vector DMA
    queues; y chunks (geometric sizes) stream on the gpsimd queue.
  - Remaining 24 m-tiles run m-major: one 1MB x DMA per m-tile (8KB/partition
    lines), 32 accumulating matmuls into one PSUM bank, epilogue scale
    (0.199*0.0215) on alternating Scalar/Vector engines, DMA out.
  - Host concatenates the 8 [4096, 512] core outputs. Exactness: bf16 holds
    these integer ranges exactly; PE multiplies exactly and accumulates in
    fp32, so only fp32 rounding remains (~1e-7 vs the fp32 reference).
"""

import numpy as np
import ml_dtypes

M = 4096
K = 4096
N = 4096
NCORES = 8
P = 128
NSH = N // NCORES  # 512 columns per core
MT = M // P        # 32 m-tiles
KT = K // P        # 32 k-tiles

G = 8                          # m-tiles in the k-major startup group
XC0 = [8, 8, 16]               # k-tiles per x chunk within the startup group
YCH = [2, 2, 4, 8, 8, 8]       # k-tiles per y chunk
N_WARMUP_MM = 7                # throwaway matmuls to trip the HAM clock gate

ZP_X = 65.0
ZP_Y = 160.0
# Match the reference's fp32 scale arithmetic as closely as possible.
SCALE = float(np.float32(0.199) * np.float32(0.0215))

_CACHE = {}


def build_nc():
    """Build + compile the per-core Bass graph (identical on all 8 cores)."""
    from concourse import bass, bacc, tile, mybir

    assert sum(YCH) == KT and sum(XC0) == KT

    nc = bacc.Bacc("TRN2", target_bir_lowering=False, debug=False)
    bf16 = mybir.dt.bfloat16
    f32 = mybir.dt.float32

    # x packed as [mt, p=k%128, kt*128+m] -> contiguous per partition row
    x_d = nc.dram_tensor("x", [MT, P, K], bf16, kind="ExternalInput").ap()
    # y shard packed as [p=k%128, kt*512+n], shipped as uint8 (half the
    # startup DMA bytes); converted to zero-point-shifted bf16 on VectorE.
    u8 = mybir.dt.uint8
    y_d = nc.dram_tensor("y", [P, KT * NSH], u8, kind="ExternalInput").ap()
    # out as [mt, m, n]
    o_d = nc.dram_tensor("out", [MT, P, NSH], f32, kind="ExternalOutput").ap()

    with tile.TileContext(nc) as tc:
        with (
            tc.tile_pool(name="wpool", bufs=1) as wpool,
            tc.tile_pool(name="ypool", bufs=1) as ypool,
            tc.tile_pool(name="x0pool", bufs=1) as x0pool,
            tc.tile_pool(name="xpool", bufs=4) as xpool,
            tc.tile_pool(name="opool", bufs=6) as opool,
            tc.tile_pool(name="ppool", bufs=8, space=bass.MemorySpace.PSUM) as ppool,
        ):
            # PE warm-up: one zeroed tile serves as both operands; the PSUM
            # bank is overwritten (start=True) by real work later and never
            # read meanwhile.
            w = wpool.tile([P, NSH], bf16, name="w")
            nc.gpsimd.memset(w[:], 0.0)
            wps = ppool.tile([P, NSH], f32, name="wps", tag="ps")
            for _ in range(N_WARMUP_MM):
                nc.tensor.matmul(wps[:], w[:, :P], w[:], start=True, stop=True)

            # y chunks (geometric sizes) on the gpsimd DMA queue; DVE does
            # the uint8 -> (v - 160) bf16 dequant (exact for these integers)
            y_ts = []
            base = 0
            for ci, ckt in enumerate(YCH):
                tu = ypool.tile([P, ckt * NSH], u8, name=f"yu{ci}", tag=f"yu{ci}")
                nc.gpsimd.dma_start(
                    tu[:], y_d[:, base * NSH:(base + ckt) * NSH]
                )
                t = ypool.tile([P, ckt * NSH], bf16, name=f"y{ci}", tag=f"y{ci}")
                nc.vector.tensor_scalar_add(t[:], tu[:], -ZP_Y)
                y_ts.append((base, ckt, t))
                base += ckt

            def y_slice(kt):
                for k0, ckt, t in y_ts:
                    if k0 <= kt < k0 + ckt:
                        return t[:, (kt - k0) * NSH:(kt - k0 + 1) * NSH]
                raise AssertionError(kt)

            # --- startup group: m-tiles 0..G-1, k-major across G PSUM banks ---
            xg0 = [[] for _ in range(G)]
            xbase = 0
            for ci, ckt in enumerate(XC0):
                for m in range(G):
                    t = x0pool.tile(
                        [P, ckt * P], bf16, name=f"x0_{m}_{ci}", tag=f"x0_{m}_{ci}"
                    )
                    eng = nc.sync if m % 2 == 0 else nc.scalar
                    eng.dma_start(
                        t[:], x_d[m][:, xbase * P:(xbase + ckt) * P]
                    )
                    xg0[m].append((xbase, ckt, t))
                xbase += ckt

            def x0_slice(m, kt):
                for k0, ckt, t in xg0[m]:
                    if k0 <= kt < k0 + ckt:
                        return t[:, (kt - k0) * P:(kt - k0 + 1) * P]
                raise AssertionError((m, kt))

            ps0 = [ppool.tile([P, NSH], f32, name="ps", tag="ps") for _ in range(G)]
            for kt in range(KT):
                for m in range(G):
                    nc.tensor.matmul(
                        ps0[m][:],
                        x0_slice(m, kt),
                        y_slice(kt),
                        start=(kt == 0),
                        stop=(kt == KT - 1),
                    )

            def epilogue(mt, ps_tile):
                o_t = opool.tile([P, NSH], f32, name="o_t", tag="o_t")
                if mt % 2 == 0:
                    nc.scalar.mul(o_t[:], ps_tile[:], SCALE)
                else:
                    nc.vector.tensor_scalar_mul(o_t[:], ps_tile[:], SCALE)
                nc.sync.dma_start(o_d[mt], o_t[:])

            for m in range(G):
                epilogue(m, ps0[m])

            # --- steady state: m-tiles G..MT-1, m-major ---
            for mt in range(G, MT):
                x_t = xpool.tile([P, K], bf16, name="x_t", tag="x_t")
                eng = nc.sync if mt % 2 == 0 else nc.scalar
                eng.dma_start(x_t[:], x_d[mt])
                ps = ppool.tile([P, NSH], f32, name="ps", tag="ps")
                for kt in range(KT):
                    nc.tensor.matmul(
                        ps[:],
                        x_t[:, kt * P:(kt + 1) * P],
                        y_slice(kt),
                        start=(kt == 0),
                        stop=(kt == KT - 1),
                    )
                epilogue(mt, ps)

    nc.compile()
    return nc


def prep_in_maps(x, y):
    """Shift zero-points, cast to bf16 (exact for these integer ranges), and
    pack for partition-contiguous DMA. Returns one in_map per core."""
    bf16 = ml_dtypes.bfloat16
    x = np.asarray(x)
    y = np.asarray(y)

    xd = (x.astype(np.float32) - np.float32(ZP_X)).astype(bf16)  # [M, K]
    # [mt, m, kt, p] -> [mt, p, kt, m]
    xp = np.ascontiguousarray(
        xd.reshape(MT, P, KT, P).transpose(0, 3, 2, 1)
    ).reshape(MT, P, K)

    # y ships as raw uint8; the kernel subtracts the zero point on-device
    yp = y.astype(np.uint8).reshape(KT, P, N).transpose(1, 0, 2)  # [p, kt, n]

    in_maps = []
    for c in range(NCORES):
        ysh = np.ascontiguousarray(yp[:, :, c * NSH:(c + 1) * NSH]).reshape(
            P, KT * NSH
        )
        in_maps.append({"x": xp, "y": ysh})
    return in_maps


def assemble_output(results):
    cols = [np.asarray(r["out"], dtype=np.float32).reshape(M, NSH) for r in results]
    return np.concatenate(cols, axis=1)


def get_nc():
    if "nc" not in _CACHE:
        _CACHE["nc"] = build_nc()
    return _CACHE["nc"]


def kernel(x, y):
    from concourse.bass_utils import run_bass_kernel_spmd

    nc = get_nc()
    in_maps = prep_in_maps(x, y)
    res = run_bass_kernel_spmd(nc, in_maps, core_ids=list(range(NCORES)))
    out = assemble_output(res.results)
    if np.isnan(out).any():
        # Cold-start insurance: a fresh device stack once produced NaN on the
        # very first execution; a retry has always been clean.
        res = run_bass_kernel_spmd(nc, in_maps, core_ids=list(range(NCORES)))
        out = assemble_output(res.results)
    return out


# revision 14
# speedup vs baseline: 1.2203x; 1.0456x over previous
"""Fused dequant-GEMM (quint8 affine) on 8 TRN2 NeuronCores.

out = ((x - 65) * 0.199) @ ((y - 160) * 0.0215),  x,y: [4096, 4096] uint8-valued int32.

Strategy (tensor-parallel, per sharding hint):
  - Shard y column-wise into 8 blocks of 512; replicate x. No collectives.
  - Host pre-packs both operands as zero-point-shifted bf16 (integers in
    [-160, 190] are exact in bf16), laid out so DMA lines are contiguous per
    partition and SBUF tiles are directly consumable as matmul operands
    (K on the partition axis).
  - Startup is latency-critical: a small throwaway matmul burst trips the PE
    HAM clock gate toward 2.4GHz while the first data DMAs land. The first 8
    m-tiles run k-major across all 8 PSUM banks, so the combined x+y stream
    is consumed at ~220 GB/s -- safely under the DMA ramp rate. x chunks
    (4 k-tiles each) are issued alternately on the sync and vector DMA
    queues; y chunks (geometric sizes) stream on the gpsimd queue.
  - Remaining 24 m-tiles run m-major: one 1MB x DMA per m-tile (8KB/partition
    lines), 32 accumulating matmuls into one PSUM bank, epilogue scale
    (0.199*0.0215) on alternating Scalar/Vector engines, DMA out.
  - Host concatenates the 8 [4096, 512] core outputs. Exactness: bf16 holds
    these integer ranges exactly; PE multiplies exactly and accumulates in
    fp32, so only fp32 rounding remains (~1e-7 vs the fp32 reference).
"""

import numpy as np
import ml_dtypes

M = 4096
K = 4096
N = 4096
NCORES = 8
P = 128
NSH = N // NCORES  # 512 columns per core
MT = M // P        # 32 m-tiles
KT = K // P        # 32 k-tiles

G = 8                          # m-tiles in the k-major startup group
XC0 = [2, 2, 4, 8, 16]         # k-tiles per x level within the startup group
N_WARMUP_EXTRA = 2             # dummies appended to bridge to first x arrival
YCH = [1, 1, 2, 4, 8, 8, 8]    # k-tiles per y chunk
N_WARMUP_MM = 7                # throwaway matmuls to trip the HAM clock gate

ZP_X = 65.0
ZP_Y = 160.0
# Match the reference's fp32 scale arithmetic as closely as possible.
SCALE = float(np.float32(0.199) * np.float32(0.0215))

_CACHE = {}


def build_nc():
    """Build + compile the per-core Bass graph (identical on all 8 cores)."""
    from concourse import bass, bacc, tile, mybir

    assert sum(YCH) == KT and sum(XC0) == KT

    nc = bacc.Bacc("TRN2", target_bir_lowering=False, debug=False)
    bf16 = mybir.dt.bfloat16
    f32 = mybir.dt.float32

    # x packed as [mt, p=k%128, kt*128+m] -> contiguous per partition row
    x_d = nc.dram_tensor("x", [MT, P, K], bf16, kind="ExternalInput").ap()
    # startup-group x: level-packed so each level is ONE wide DMA covering
    # all 8 m-tiles: [p, level(m, kt_in_level, mcol)]
    x0_d = nc.dram_tensor("x0", [P, G * K], bf16, kind="ExternalInput").ap()
    # y shard packed as [p=k%128, kt*512+n], shipped as uint8 (half the
    # startup DMA bytes); converted to zero-point-shifted bf16 on VectorE.
    u8 = mybir.dt.uint8
    y_d = nc.dram_tensor("y", [P, KT * NSH], u8, kind="ExternalInput").ap()
    # out as [mt, m, n]
    o_d = nc.dram_tensor("out", [MT, P, NSH], f32, kind="ExternalOutput").ap()

    with tile.TileContext(nc) as tc:
        with (
            tc.tile_pool(name="wpool", bufs=1) as wpool,
            tc.tile_pool(name="ypool", bufs=1) as ypool,
            tc.tile_pool(name="x0pool", bufs=1) as x0pool,
            tc.tile_pool(name="xpool", bufs=4) as xpool,
            tc.tile_pool(name="opool", bufs=6) as opool,
            tc.tile_pool(name="ppool", bufs=8, space=bass.MemorySpace.PSUM) as ppool,
        ):
            # PE warm-up: one zeroed tile serves as both operands; the PSUM
            # bank is overwritten (start=True) by real work later and never
            # read meanwhile.
            w = wpool.tile([P, NSH], bf16, name="w")
            nc.gpsimd.memset(w[:], 0.0)
            wps = ppool.tile([P, NSH], f32, name="wps", tag="ps")
            for _ in range(N_WARMUP_MM + N_WARMUP_EXTRA):
                nc.tensor.matmul(wps[:], w[:, :P], w[:], start=True, stop=True)

            # All input DMAs ride ONE queue (sync) in exact consumption
            # order -- per-queue FIFO is the only priority control we have,
            # and concurrent queues would split the ~358 GB/s HBM pipe away
            # from the critical startup stream.
            y_state = {"ci": 0, "base": 0}
            y_ts = []

            def emit_y_chunk():
                ci, base = y_state["ci"], y_state["base"]
                ckt = YCH[ci]
                tu = ypool.tile([P, ckt * NSH], u8, name=f"yu{ci}", tag=f"yu{ci}")
                nc.sync.dma_start(tu[:], y_d[:, base * NSH:(base + ckt) * NSH])
                t = ypool.tile([P, ckt * NSH], bf16, name=f"y{ci}", tag=f"y{ci}")
                nc.vector.tensor_scalar_add(t[:], tu[:], -ZP_Y)
                y_ts.append((base, ckt, t))
                y_state["ci"] = ci + 1
                y_state["base"] = base + ckt

            def y_slice(kt):
                for k0, ckt, t in y_ts:
                    if k0 <= kt < k0 + ckt:
                        return t[:, (kt - k0) * NSH:(kt - k0 + 1) * NSH]
                raise AssertionError(kt)

            # --- startup group: m-tiles 0..G-1, k-major across G PSUM banks ---
            # one wide DMA per x level (all 8 m-tiles together)
            xlv = []
            x_state = {"ci": 0, "base": 0, "off": 0}

            def emit_x_level():
                ci, xbase, off = x_state["ci"], x_state["base"], x_state["off"]
                ckt = XC0[ci]
                t = x0pool.tile(
                    [P, G * ckt * P], bf16, name=f"x0l{ci}", tag=f"x0l{ci}"
                )
                nc.sync.dma_start(t[:], x0_d[:, off:off + G * ckt * P])
                xlv.append((xbase, ckt, t))
                x_state["ci"] = ci + 1
                x_state["base"] = xbase + ckt
                x_state["off"] = off + G * ckt * P

            # need-order: y0, L0a, y1, L0b, y2, L1, y3, L2, y4, L3, y5, y6
            emit_y_chunk(); emit_x_level()          # kt0: y0, x kt0-1
            emit_y_chunk(); emit_x_level()          # kt1, x kt2-3
            emit_y_chunk(); emit_x_level()          # kt2-3, x kt4-7
            emit_y_chunk(); emit_x_level()          # kt4-7, x kt8-15
            emit_y_chunk(); emit_x_level()          # kt8-15, x kt16-31
            emit_y_chunk(); emit_y_chunk()          # kt16-23, kt24-31
            assert x_state["ci"] == len(XC0) and y_state["ci"] == len(YCH)

            def x0_slice(m, kt):
                for k0, ckt, t in xlv:
                    if k0 <= kt < k0 + ckt:
                        return t[:, (m * ckt + (kt - k0)) * P:(m * ckt + (kt - k0) + 1) * P]
                raise AssertionError((m, kt))

            ps0 = [ppool.tile([P, NSH], f32, name="ps", tag="ps") for _ in range(G)]
            for kt in range(KT):
                for m in range(G):
                    nc.tensor.matmul(
                        ps0[m][:],
                        x0_slice(m, kt),
                        y_slice(kt),
                        start=(kt == 0),
                        stop=(kt == KT - 1),
                    )

            def epilogue(mt, ps_tile):
                o_t = opool.tile([P, NSH], f32, name="o_t", tag="o_t")
                if mt % 2 == 0:
                    nc.scalar.mul(o_t[:], ps_tile[:], SCALE)
                else:
                    nc.vector.tensor_scalar_mul(o_t[:], ps_tile[:], SCALE)
                nc.scalar.dma_start(o_d[mt], o_t[:])

            for m in range(G):
                epilogue(m, ps0[m])

            # --- steady state: m-tiles G..MT-1, m-major ---
            for mt in range(G, MT):
                x_t = xpool.tile([P, K], bf16, name="x_t", tag="x_t")
                nc.sync.dma_start(x_t[:], x_d[mt])
                if mt < MT - 1:
                    ps = ppool.tile([P, NSH], f32, name="ps", tag="ps")
                    for kt in range(KT):
                        nc.tensor.matmul(
                            ps[:],
                            x_t[:, kt * P:(kt + 1) * P],
                            y_slice(kt),
                            start=(kt == 0),
                            stop=(kt == KT - 1),
                        )
                    epilogue(mt, ps)
                else:
                    # final m-tile: two N-halves so half 0's epilogue + DMA
                    # hide under half 1's matmuls, halving the exposed tail
                    H = NSH // 2
                    for h in range(2):
                        psh = ppool.tile([P, H], f32, name="psh", tag="ps")
                        for kt in range(KT):
                            nc.tensor.matmul(
                                psh[:],
                                x_t[:, kt * P:(kt + 1) * P],
                                y_slice(kt)[:, h * H:(h + 1) * H],
                                start=(kt == 0),
                                stop=(kt == KT - 1),
                            )
                        o_t = opool.tile([P, H], f32, name="o_th", tag="o_th")
                        nc.vector.tensor_scalar_mul(o_t[:], psh[:], SCALE)
                        nc.scalar.dma_start(o_d[mt][:, h * H:(h + 1) * H], o_t[:])

    nc.compile()
    return nc


def prep_in_maps(x, y):
    """Shift zero-points, cast to bf16 (exact for these integer ranges), and
    pack for partition-contiguous DMA. Returns one in_map per core."""
    bf16 = ml_dtypes.bfloat16
    x = np.asarray(x)
    y = np.asarray(y)

    xd = (x.astype(np.float32) - np.float32(ZP_X)).astype(bf16)  # [M, K]
    # [mt, m, kt, p] -> [mt, p, kt, m]
    xq = xd.reshape(MT, P, KT, P)
    xp = np.ascontiguousarray(xq.transpose(0, 3, 2, 1)).reshape(MT, P, K)
    # startup-group x, level-packed: [p, (m, kt_in_level, mcol)] per level
    lvls = []
    base = 0
    for ckt in XC0:
        lvls.append(
            xq[0:G, :, base:base + ckt, :].transpose(3, 0, 2, 1).reshape(P, G * ckt * P)
        )
        base += ckt
    x0p = np.ascontiguousarray(np.concatenate(lvls, axis=1))

    # y ships as raw uint8; the kernel subtracts the zero point on-device
    yp = y.astype(np.uint8).reshape(KT, P, N).transpose(1, 0, 2)  # [p, kt, n]

    in_maps = []
    for c in range(NCORES):
        ysh = np.ascontiguousarray(yp[:, :, c * NSH:(c + 1) * NSH]).reshape(
            P, KT * NSH
        )
        in_maps.append({"x": xp, "x0": x0p, "y": ysh})
    return in_maps


def assemble_output(results):
    cols = [np.asarray(r["out"], dtype=np.float32).reshape(M, NSH) for r in results]
    return np.concatenate(cols, axis=1)


def get_nc():
    if "nc" not in _CACHE:
        _CACHE["nc"] = build_nc()
    return _CACHE["nc"]


def _run(nc, in_maps):
    from concourse.bass_utils import run_bass_kernel_spmd

    res = run_bass_kernel_spmd(nc, in_maps, core_ids=list(range(NCORES)))
    return assemble_output(res.results)


def kernel(x, y):
    import time

    nc = get_nc()
    in_maps = prep_in_maps(x, y)
    try:
        out = _run(nc, in_maps)
    except Exception:
        # A wedged device (NRT_EXEC_UNIT_UNRECOVERABLE) recovers on retry.
        time.sleep(10)
        out = _run(nc, in_maps)
    if np.isnan(out).any():
        # Cold-start insurance: a fresh device stack once produced NaN on the
        # very first execution; a retry has always been clean.
        out = _run(nc, in_maps)
    return out
